# revision 7
# baseline (speedup 1.0000x reference)
"""MoE (8 experts, top-2, sigmoid router, SwiGLU + shared expert) on 8 TRN2 cores.

Strategy: expert-parallel with host-side dispatch. The router (sigmoid scores,
top-2, combine weights) runs on the host in fp32 numpy — verified to match the
jax reference bit-for-bit on expert selection (min 2nd-vs-3rd score gap 1.3e-4
vs ~1e-6 matmul noise). Tokens are gathered per expert, pre-scaled by their
combine weight (silu(s*g)*(s*u) == silu(W1(s*x))*(W3(s*x))), padded to a fixed
capacity C, and dispatched: core e runs a dense SwiGLU for expert e over its
<=C tokens plus the shared expert over a 256-token shard. This cuts device
FLOPs 2.8x vs dense all-experts (top-2 of 8 + shared). Activations are
computed directly in [hidden, token] layout so the down-projection needs no
transposes; weights are pre-tiled on host so every DMA is a single
contiguous >=2KB-per-partition transfer. The host scatter-adds the two expert
contributions per token and adds the shared output.
"""
import numpy as np
import ml_dtypes

import concourse.bass as bass
import concourse.tile as tile
from concourse import bacc, mybir
from concourse.bass_utils import run_bass_kernel_spmd

P = 128
N_CORES = 8
SLEN = 2048
DIM = 2048
HID = 1024
E = 8
TOP_K = 2
SSH = SLEN // N_CORES          # shared-expert tokens per core
DC = DIM // P                  # 16 contraction chunks over dim
HC = HID // P                  # 8 chunks over hidden
TCW = 512                      # max token chunk width (one fp32 PSUM bank)
BF16 = mybir.dt.bfloat16
F32 = mybir.dt.float32
DEF_C = 548                    # routed-token capacity per expert

_CACHE: dict = {}


def _chunks(T):
    # balanced chunks <= TCW (avoids tiny SEQ-bound tail matmuls)
    n = -(-T // TCW)
    base, rem = divmod(T, n)
    out, t0 = [], 0
    for i in range(n):
        w = base + (1 if i < rem else 0)
        out.append((t0, w))
        t0 += w
    return out


def _build(C):
    nc = bacc.Bacc("TRN2", target_bir_lowering=False, debug=False,
                   num_devices=N_CORES)

    # x layouts: [p, dc, t] with dim = dc*128 + p
    xr_d = nc.dram_tensor("xr", [P, DC, C], BF16, kind="ExternalInput").ap()
    xs_d = nc.dram_tensor("xs", [P, DC, SSH], BF16, kind="ExternalInput").ap()
    # up/gate weights [ht, p, dc, h]: lhsT chunks [128 dim, 128 hid]
    wg_d = nc.dram_tensor("wg", [HC, P, DC, P], BF16, kind="ExternalInput").ap()
    wu_d = nc.dram_tensor("wu", [HC, P, DC, P], BF16, kind="ExternalInput").ap()
    swg_d = nc.dram_tensor("swg", [HC, P, DC, P], BF16, kind="ExternalInput").ap()
    swu_d = nc.dram_tensor("swu", [HC, P, DC, P], BF16, kind="ExternalInput").ap()
    # down weights [dt, p, hc, d]: lhsT chunks [128 hid, 128 dim]
    wd_d = nc.dram_tensor("wd", [DC, P, HC, P], BF16, kind="ExternalInput").ap()
    swd_d = nc.dram_tensor("swd", [DC, P, HC, P], BF16, kind="ExternalInput").ap()
    # outputs [dt, d, t] with dim = dt*128 + d
    yr_d = nc.dram_tensor("yr", [DC, P, C], F32, kind="ExternalOutput").ap()
    ys_d = nc.dram_tensor("ys", [DC, P, SSH], F32, kind="ExternalOutput").ap()

    # shared branch first: PE start is gated on only xs (1MB) + swg[0]
    # (0.5MB) arriving, on separate DMA queues.
    branches = [
        (SSH, xs_d, swg_d, swu_d, swd_d, ys_d),
        (C, xr_d, wg_d, wu_d, wd_d, yr_d),
    ]

    # weights go on the SP DGE queue (nc.sync); x loads and y stores on the
    # Activation DGE queue (nc.scalar) so output traffic never blocks the
    # weight prefetch stream.
    with tile.TileContext(nc) as tc:
        with tc.tile_pool(name="xpool", bufs=1) as xpool, \
             tc.tile_pool(name="hpool", bufs=1) as hpool, \
             tc.tile_pool(name="wpool", bufs=4) as wpool, \
             tc.tile_pool(name="wdpool", bufs=6) as wdpool, \
             tc.tile_pool(name="upsum", bufs=2, space="PSUM") as upsum, \
             tc.tile_pool(name="dpsum", bufs=3, space="PSUM") as dpsum, \
             tc.tile_pool(name="tmp", bufs=3) as tmp, \
             tc.tile_pool(name="ypool", bufs=3) as ypool:

            xt = {}
            for bi, (T, x_d, *_r) in enumerate(branches):
                xt[bi] = xpool.tile([P, DC, T], BF16, tag=f"x{bi}",
                                    name=f"x{bi}")
                nc.scalar.dma_start(xt[bi][:], x_d[:])

            for bi, (T, x_d, g_d, u_d, d_d, y_d) in enumerate(branches):
                h = hpool.tile([P, HC, T], BF16, tag=f"h{bi}")
                # ---- up/gate: pg/pu[hid, tok] accumulated over dim chunks
                for ht in range(HC):
                    wg = wpool.tile([P, DC, P], BF16, tag="wg")
                    wu = wpool.tile([P, DC, P], BF16, tag="wu")
                    nc.sync.dma_start(wg[:], g_d[ht])
                    nc.sync.dma_start(wu[:], u_d[ht])
                    for (t0, tw) in _chunks(T):
                        pg = upsum.tile([P, TCW], F32, tag="pg")
                        pu = upsum.tile([P, TCW], F32, tag="pu")
                        for dc in range(DC):
                            st, sp = (dc == 0), (dc == DC - 1)
                            nc.tensor.matmul(pg[:, :tw], wg[:, dc, :],
                                             xt[bi][:, dc, t0:t0 + tw],
                                             start=st, stop=sp)
                            nc.tensor.matmul(pu[:, :tw], wu[:, dc, :],
                                             xt[bi][:, dc, t0:t0 + tw],
                                             start=st, stop=sp)
                        sg = tmp.tile([P, TCW], BF16, tag="sg")
                        su = tmp.tile([P, TCW], BF16, tag="su")
                        nc.scalar.activation(sg[:, :tw], pg[:, :tw],
                                             mybir.ActivationFunctionType.Silu)
                        nc.vector.tensor_copy(su[:, :tw], pu[:, :tw])
                        nc.vector.tensor_mul(h[:, ht, t0:t0 + tw],
                                             sg[:, :tw], su[:, :tw])
                # ---- down: py[dim, tok] accumulated over hidden chunks
                for dt in range(DC):
                    wd = wdpool.tile([P, HC, P], BF16, tag="wd")
                    nc.sync.dma_start(wd[:], d_d[dt])
                    for (t0, tw) in _chunks(T):
                        py = dpsum.tile([P, TCW], F32, tag="py")
                        for hc in range(HC):
                            nc.tensor.matmul(py[:, :tw], wd[:, hc, :],
                                             h[:, hc, t0:t0 + tw],
                                             start=(hc == 0), stop=(hc == HC - 1))
                        yt = ypool.tile([P, TCW], F32, tag="yt")
                        nc.scalar.copy(yt[:, :tw], py[:, :tw])
                        nc.scalar.dma_start(y_d[dt, :, t0:t0 + tw], yt[:, :tw])

    nc.compile()
    return nc


def _get_nc(C=None):
    if C is None:
        C = _CACHE.get("last_C", DEF_C)
    if ("nc", C) not in _CACHE:
        _CACHE[("nc", C)] = _build(C)
    _CACHE["last_C"] = C
    return _CACHE[("nc", C)]


def _bf16(a):
    return np.ascontiguousarray(a.astype(ml_dtypes.bfloat16))


def _wg_layout(w):
    # w: [HID, DIM] -> [ht, p, dc, h]
    return np.ascontiguousarray(
        w.reshape(HC, P, DC, P).transpose(0, 3, 2, 1))


def _wd_layout(w):
    # w: [DIM, HID] -> [dt, p, hc, d]
    return np.ascontiguousarray(
        w.reshape(DC, P, HC, P).transpose(0, 3, 2, 1))


def _x_layout(rows, T):
    # rows: [n, DIM] bf16 -> [p, dc, t] padded to T tokens
    arr = np.zeros((T, DIM), dtype=ml_dtypes.bfloat16)
    arr[:rows.shape[0]] = rows
    return np.ascontiguousarray(arr.reshape(T, DC, P).transpose(2, 1, 0))


def kernel(x, gate, expert_bias, w1, w2, w3, sw1, sw2, sw3, _want_results=False):
    x = np.asarray(x, dtype=np.float32)
    gate = np.ascontiguousarray(np.asarray(gate, dtype=np.float32))
    expert_bias = np.asarray(expert_bias, dtype=np.float32)
    w1 = np.asarray(w1, dtype=np.float32)
    w2 = np.asarray(w2, dtype=np.float32)
    w3 = np.asarray(w3, dtype=np.float32)

    xt = x.reshape(SLEN, DIM)

    # ---- host router (fp32, matches jax top-2 selection on this regime)
    logits = xt @ gate
    scores = 1.0 / (1.0 + np.exp(-logits))
    v = scores + expert_bias[None, :]
    top2 = np.argpartition(-v, TOP_K - 1, axis=1)[:, :TOP_K]      # unordered
    s_top = np.take_along_axis(scores, top2, axis=1)

    e_flat = top2.ravel()
    tok_flat = np.repeat(np.arange(SLEN), TOP_K)
    s_flat = s_top.ravel()
    order = np.argsort(e_flat, kind="stable")
    counts = np.bincount(e_flat, minlength=E)
    offs = np.concatenate([[0], np.cumsum(counts)])

    C = max(DEF_C, int(-(-counts.max() // 4) * 4))  # DEF_C covers max 545

    # pre-scaled routed tokens, expert-sorted
    xs_rows = (xt[tok_flat[order]] * s_flat[order][:, None]).astype(
        ml_dtypes.bfloat16)

    # ---- per-core inputs
    wg_all = [_wg_layout(_bf16(w1[e])) for e in range(E)]
    wu_all = [_wg_layout(_bf16(w3[e])) for e in range(E)]
    wd_all = [_wd_layout(_bf16(w2[e])) for e in range(E)]
    swg = _wg_layout(_bf16(np.asarray(sw1, np.float32)))
    swu = _wg_layout(_bf16(np.asarray(sw3, np.float32)))
    swd = _wd_layout(_bf16(np.asarray(sw2, np.float32)))

    in_maps = []
    for c in range(N_CORES):
        rows = xs_rows[offs[c]:offs[c + 1]]
        xr = _x_layout(rows, C)
        xsh = _x_layout(_bf16(xt[c * SSH:(c + 1) * SSH]), SSH)
        in_maps.append({
            "xr": xr, "xs": xsh,
            "wg": wg_all[c], "wu": wu_all[c], "wd": wd_all[c],
            "swg": swg, "swu": swu, "swd": swd,
        })

    nc = _get_nc(C)
    res = run_bass_kernel_spmd(nc, in_maps, list(range(N_CORES)))

    # ---- host combine
    routed_rows = np.empty((SLEN * TOP_K, DIM), dtype=np.float32)
    shared = np.empty((SLEN, DIM), dtype=np.float32)
    for c in range(N_CORES):
        yr = res.results[c]["yr"]                   # [DC, P, C]
        ys = res.results[c]["ys"]                   # [DC, P, SSH]
        n = counts[c]
        routed_rows[order[offs[c]:offs[c + 1]]] = \
            yr.transpose(2, 0, 1).reshape(C, DIM)[:n]
        shared[c * SSH:(c + 1) * SSH] = ys.transpose(2, 0, 1).reshape(SSH, DIM)

    routed = routed_rows.reshape(SLEN, TOP_K, DIM).sum(axis=1)
    out = (routed + shared).reshape(1, 1, SLEN, DIM).astype(np.float32)
    if _want_results:
        return out, res
    return out
